# revision 18
# baseline (speedup 1.0000x reference)
"""Trainium2 Bass kernel for nn_Blur: depthwise 4x4 FIR blur (upfirdn2d pad=(2,1)).

Full inputs in, full output out. Internally shards the 4096 (b,c) images
across 8 NeuronCores (pure data parallel, no collectives).

v2 (bf16): tolerance is rel_err < 2e-2, so all device I/O is bf16 (host-side
RNE cast) — halves HBM traffic vs fp32 (memory-regime roofline win).  The
host also pre-arranges x into the exact SBUF layout the kernel wants
([h, img-major cols] with 2 zero gap columns per image), so every DMA is a
single large per-partition-contiguous transfer (~1.6 MB, 12.5 KB/partition
descriptors) instead of many 512 B-chunk strided ones.

Compute per core (512 images of [H=128, W=128]): the 4x4 depthwise conv
factors into 4 column-convolutions along H, each a banded matmul with the
contraction over the partition (H) axis, with the W-shift (j-2) realized as
a shifted moving-operand read of an accumulating matmul:
  psum[:, c] += W_j^T @ x[:, c + (j-2)]     W_j[hi, ho] = wf[hi-ho+2, j]
Images are packed at stride 130 (2 zero gap cols) so shifted reads pick up
zero padding at image edges.  Groups of 3 images share a PSUM bank; 4
groups (4 banks) form one chunk tile so PSUM->SBUF evacuation is one big
Vector/Scalar copy per chunk.
"""

import os
import sys
from contextlib import ExitStack

for _p in ("/opt/trn_rl_repo", "/root/.axon_site/_ro/trn_rl_repo"):
    if os.path.isdir(_p) and _p not in sys.path:
        sys.path.append(_p)

import ml_dtypes
import numpy as np

import concourse.bass as bass  # noqa: F401  (engine types referenced via nc)
import concourse.tile as tile
from concourse import bacc, bass_utils, mybir

BF16 = np.dtype(ml_dtypes.bfloat16)

B, C, H, W = 16, 256, 128, 128
N_CORES = 8
GROUP = 3          # images per PSUM bank / matmul group
STRIDE = 130       # 2-col gap + 128 data cols per image in the packed layout
PAD0 = 2           # upfirdn2d pad before (both spatial dims)
TILE = 24          # images per DMA tile (must be multiple of GROUP)
QG = 4             # PSUM banks (groups) per chunk tile
OFFLOAD_FRAC = 0.5  # fraction of full tiles W-conv'd on Vector/GpSimd (separable)

_PROGRAM_CACHE: dict[object, object] = {}


def _band_matrices(kern: np.ndarray) -> np.ndarray:
    """bands[j][hi, ho] = wf[hi-ho+2, j], wf = flip(kern). Shape [4,128,128]."""
    wf = np.flip(np.asarray(kern, dtype=np.float64), (0, 1))
    bands = np.zeros((4, H, H), dtype=np.float64)
    ho = np.arange(H)
    for j in range(4):
        for i in range(4):
            d = i - PAD0            # hi - ho
            hi = ho + d
            m = (hi >= 0) & (hi < H)
            bands[j][hi[m], ho[m]] = wf[i, j]
    return np.ascontiguousarray(bands.astype(np.float32))


def _tiles(n_images: int):
    """Split n_images into DMA tiles of at most TILE images.

    The first tiles ramp up small so the first matmul can start as soon as
    a small DMA lands; any ragged remainder goes LAST so the final
    output DMA (pure tail latency) is as small as possible.
    """
    ramp = [6, 6, 12]
    out = []
    i = 0
    for r in ramp:
        if n_images - i > r:
            out.append((i, r))
            i += r
    while i < n_images:
        n = min(TILE, n_images - i)
        out.append((i, n))
        i += n
    return out


def _groups(n_images: int):
    """Split a tile's images into matmul groups of at most GROUP, avoiding a
    trailing 1-image group (rebalance 3+1 -> 2+2)."""
    out = []
    i = 0
    while i < n_images:
        n = min(GROUP, n_images - i)
        out.append((i, n))
        i += n
    if len(out) >= 2 and out[-1][1] == 1:
        i0, n0 = out[-2]
        out[-2] = (i0, 2)
        out[-1] = (i0 + 2, 2)
    return out


def _offload_tiles(tiles, taps):
    """Pick which full tiles get the separable W-conv offload."""
    if taps is None or OFFLOAD_FRAC <= 0:
        return set()
    full = [i for i, (_, tn) in enumerate(tiles) if tn == TILE]
    if not full:
        return set()
    n_off = max(1, int(round(OFFLOAD_FRAC * len(full))))
    chosen = list(full[1::2][:n_off])
    for i in full[0::2]:
        if len(chosen) >= n_off:
            break
        chosen.append(i)
    return set(chosen)


def build_program(n_images: int, taps=None, xt_bufs: int = 6):
    """Build + compile the per-core Bass program for n_images [128,128] images.

    DRAM layout (host-prepared, bf16):
      x: [H, n_images*STRIDE + 2]  image k's column w at STRIDE*k + 2 + w,
         cols {STRIDE*k, STRIDE*k+1} and the trailing 2 are zeros.
      y: [H, n_images*W]           image k's column w at W*k + w.

    bands[0:4] are the fused HxW band matrices (4-pass path); bands[4] is
    the H-only band (offload path).  taps, when not None, are the 4 W-conv
    scalars (flipped W factor of the separable kernel) baked as immediates;
    tiles in _offload_tiles then run: PE H-conv (1 pass) -> PSUM -> SBUF
    evac (Scalar) -> 4-tap W-conv FMA chain (Vector/GpSimd alternating).
    """
    nc = bacc.Bacc("TRN2", target_bir_lowering=False, debug=False)
    f32 = mybir.dt.float32
    bf16 = mybir.dt.bfloat16

    x_d = nc.dram_tensor("x", [H, n_images * STRIDE + 2], bf16, kind="ExternalInput")
    b_d = nc.dram_tensor("bands", [5, H, H], bf16, kind="ExternalInput")
    y_d = nc.dram_tensor("y", [H, n_images * W], bf16, kind="ExternalOutput")

    tiles = _tiles(n_images)
    off_tiles = _offload_tiles(tiles, taps)

    with ExitStack() as ctx:
        tc = ctx.enter_context(tile.TileContext(nc))
        wpool = ctx.enter_context(tc.tile_pool(name="wpool", bufs=1))
        xpool = ctx.enter_context(tc.tile_pool(name="xpool", bufs=xt_bufs))
        opool = ctx.enter_context(tc.tile_pool(name="opool", bufs=4))
        tapool = ctx.enter_context(tc.tile_pool(name="tapool", bufs=2))
        wkpool = ctx.enter_context(tc.tile_pool(name="wkpool", bufs=6))
        ppool = ctx.enter_context(tc.tile_pool(name="ppool", bufs=2, space="PSUM"))

        wt = wpool.tile([H, 5 * H], bf16)
        nc.sync.dma_start(
            wt.rearrange("p (j b) -> p j b", b=H), b_d.rearrange("j a b -> a j b")
        )

        # Warm up the PE HAM clock gate with dummy matmuls on the weights
        # tile while the first input DMA is in flight: real matmuls then
        # start at 2.4 GHz instead of 1.2.
        warm = ppool.tile([H, 512 * QG], f32, tag="pt", name="pt")
        for _ in range(20):
            nc.tensor.matmul(
                warm[:, 0:256], wt[:, 0:H], wt[:, 0:256], start=True, stop=True
            )

        in_dma_engines = [nc.sync, nc.gpsimd]
        xts: dict[int, object] = {}

        def emit_in_dma(ti):
            i0, tn = tiles[ti]
            xt = xpool.tile([H, tn * STRIDE + 2], bf16, tag="xt", name="xt")
            in_dma_engines[ti % 2].dma_start(
                xt, x_d[:, i0 * STRIDE : i0 * STRIDE + tn * STRIDE + 2]
            )
            xts[ti] = xt

        for ti in range(min(4, len(tiles))):
            emit_in_dma(ti)

        def tA_view(tA, d, tn):
            """[p, tn, W] view of the gap-layout tile shifted by d columns."""
            span = tn * STRIDE
            if d <= 0:
                sl = tA[:, PAD0 + d : PAD0 + d + span]
                lo = 0
            else:
                sl = tA[:, PAD0 : PAD0 + span]
                lo = d
            return sl.rearrange("p (k c) -> p k c", c=STRIDE)[:, :, lo : lo + W]

        mult = mybir.AluOpType.mult
        add = mybir.AluOpType.add

        copy_idx = 0
        wconv_idx = 0
        for ti, (i0, tn) in enumerate(tiles):
            if ti + 4 < len(tiles):
                emit_in_dma(ti + 4)
            xt = xts.pop(ti)

            if ti in off_tiles:
                # --- separable offload path: PE does only the H-conv ---
                # Every 3rd offload tile's W-conv runs on GpSimd (walrus
                # rejects scalar_tensor_tensor on Pool, so it gets a
                # mul/add chain); the rest use Vector's fused FMA.
                use_gp = wconv_idx % 3 == 2
                wconv_idx += 1
                ot = opool.tile([H, tn * W], bf16, tag="ot", name="ot")
                span = tn * STRIDE + 2
                tA = tapool.tile([H, span], bf16, tag="ta", name="ta")
                evac = nc.vector if use_gp else nc.scalar
                c0 = 0
                while c0 < span:
                    cw = min(512 * QG, span - c0)
                    pt = ppool.tile([H, 512 * QG], f32, tag="pt", name="pt")
                    s = 0
                    while s < cw:
                        w_ = min(512, cw - s)
                        nc.tensor.matmul(
                            pt[:, s : s + w_],
                            wt[:, 4 * H : 5 * H],
                            xt[:, c0 + s : c0 + s + w_],
                            start=True,
                            stop=True,
                        )
                        s += w_
                    if evac is nc.vector:
                        evac.tensor_copy(tA[:, c0 : c0 + cw], pt[:, 0:cw])
                    else:
                        evac.copy(tA[:, c0 : c0 + cw], pt[:, 0:cw])
                    c0 += cw

                v = [tA_view(tA, d, tn) for d in (-2, -1, 0, 1)]
                wk1 = wkpool.tile([H, tn * W], bf16, tag="wk", name="wk")
                wk2 = wkpool.tile([H, tn * W], bf16, tag="wk", name="wk")
                w1v = wk1.rearrange("p (k c) -> p k c", c=W)
                w2v = wk2.rearrange("p (k c) -> p k c", c=W)
                otv = ot.rearrange("p (k c) -> p k c", c=W)
                if not use_gp:
                    e = nc.vector
                    e.tensor_scalar_mul(w1v, v[0], taps[0])
                    e.scalar_tensor_tensor(w2v, v[1], taps[1], w1v, mult, add)
                    e.scalar_tensor_tensor(w1v, v[2], taps[2], w2v, mult, add)
                    e.scalar_tensor_tensor(otv, v[3], taps[3], w1v, mult, add)
                else:
                    e = nc.gpsimd
                    wk3 = wkpool.tile([H, tn * W], bf16, tag="wk", name="wk")
                    w3v = wk3.rearrange("p (k c) -> p k c", c=W)
                    sym = taps[0] == taps[3] and taps[1] == taps[2]
                    if sym:
                        e.tensor_add(w1v, v[0], v[3])
                        e.tensor_add(w2v, v[1], v[2])
                        e.tensor_scalar_mul(w3v, w1v, taps[0])
                        e.tensor_scalar_mul(w1v, w2v, taps[1])
                        e.tensor_add(otv, w3v, w1v)
                    else:
                        e.tensor_scalar_mul(w1v, v[0], taps[0])
                        e.tensor_scalar_mul(w2v, v[1], taps[1])
                        e.tensor_add(w3v, w1v, w2v)
                        e.tensor_scalar_mul(w1v, v[2], taps[2])
                        e.tensor_add(w2v, w3v, w1v)
                        e.tensor_scalar_mul(w1v, v[3], taps[3])
                        e.tensor_add(otv, w2v, w1v)

                out_eng = (nc.sync, nc.scalar)[ti % 2]
                out_eng.dma_start(y_d[:, i0 * W : (i0 + tn) * W], ot)
                continue

            gs = _groups(tn)
            chunks = [gs[s : s + QG] for s in range(0, len(gs), QG)]
            ot = opool.tile([H, tn * W], bf16, tag="ot", name="ot")

            for chunk in chunks:
                nq = len(chunk)
                pt = ppool.tile([H, 512 * nq], f32, tag="pt", name="pt")
                # j-outer order amortizes the 4 stationary (band) loads over
                # the whole chunk; j=2 (d=0) first for the full-width
                # has_written-clearing write.
                for idx, j in enumerate((2, 0, 1, 3)):
                    d = j - PAD0
                    for q, (goff, n) in enumerate(chunk):
                        a = PAD0
                        b = STRIDE * n + PAD0 - (PAD0 if d > 0 else 0)
                        base = goff * STRIDE
                        nc.tensor.matmul(
                            pt[:, 512 * q + a : 512 * q + b],
                            wt[:, H * j : H * (j + 1)],
                            xt[:, base + a + d : base + b + d],
                            start=(idx == 0),
                            stop=(idx == 3),
                        )

                # PSUM -> SBUF evacuation (fp32 -> bf16).  One strided copy
                # per chunk when the chunk is uniform (all groups GROUP-sized);
                # per-group copies otherwise (ragged tail).  Scalar unless no
                # offloading is active (then alternate with Vector).
                uniform = all(n == GROUP for _, n in chunk)
                if off_tiles:
                    eng = (nc.scalar, nc.scalar)
                else:
                    eng = (nc.vector, nc.scalar)
                if uniform:
                    psrc = (
                        pt.rearrange("p (q c) -> p q c", c=512)[
                            :, :, : GROUP * STRIDE
                        ]
                        .rearrange("p q (k c) -> p q k c", c=STRIDE)[
                            :, :, :, PAD0 : PAD0 + W
                        ]
                    )
                    odst = ot[
                        :, chunk[0][0] * W : (chunk[-1][0] + GROUP) * W
                    ].rearrange("p (q k c) -> p q k c", q=nq, c=W)
                    e = eng[copy_idx % 2]
                    if e is nc.vector:
                        e.tensor_copy(odst, psrc)
                    else:
                        e.copy(odst, psrc)
                    copy_idx += 1
                else:
                    for q, (goff, n) in enumerate(chunk):
                        psrc = pt[:, 512 * q : 512 * q + STRIDE * n].rearrange(
                            "p (k c) -> p k c", c=STRIDE
                        )[:, :, PAD0 : PAD0 + W]
                        odst = ot[:, goff * W : (goff + n) * W].rearrange(
                            "p (k c) -> p k c", c=W
                        )
                        e = eng[copy_idx % 2]
                        if e is nc.vector:
                            e.tensor_copy(odst, psrc)
                        else:
                            e.copy(odst, psrc)
                        copy_idx += 1

            out_eng = (nc.sync, nc.scalar)[ti % 2]
            out_eng.dma_start(y_d[:, i0 * W : (i0 + tn) * W], ot)

    nc.compile()
    return nc


def _get_program(n_images: int, taps=None):
    key = (n_images, taps)
    if key not in _PROGRAM_CACHE:
        _PROGRAM_CACHE[key] = build_program(n_images, taps=taps)
    return _PROGRAM_CACHE[key]


def _separable(kern: np.ndarray):
    """Return (bands5_f32, taps) — taps None when kern is not rank-1."""
    K = np.asarray(kern, dtype=np.float64)
    bands5 = np.zeros((5, H, H), dtype=np.float32)
    bands5[0:4] = _band_matrices(kern)
    U, S, Vt = np.linalg.svd(K)
    if S[1] > 1e-6 * max(S[0], 1e-30):
        return bands5, None
    a = U[:, 0] * np.sqrt(S[0])
    b = Vt[0, :] * np.sqrt(S[0])
    af = a[::-1]  # flipped H factor
    bfl = b[::-1]  # flipped W factor -> the 4 free-dim taps
    ho = np.arange(H)
    Bh = np.zeros((H, H), dtype=np.float64)
    for i in range(4):
        hi = ho + (i - PAD0)
        m = (hi >= 0) & (hi < H)
        Bh[hi[m], ho[m]] = af[i]
    bands5[4] = Bh.astype(np.float32)
    taps = tuple(float(np.float32(v)) for v in bfl)
    return bands5, taps


def _pack_input(xc_bf16: np.ndarray) -> np.ndarray:
    """[n, H, W] bf16 -> [H, n*STRIDE + 2] bf16 gap layout."""
    n = xc_bf16.shape[0]
    arr = np.zeros((H, n * STRIDE + 2), dtype=BF16)
    v = np.lib.stride_tricks.as_strided(
        arr,
        shape=(H, n, STRIDE),
        strides=(arr.strides[0], STRIDE * arr.itemsize, arr.itemsize),
    )
    v[:, :, PAD0:] = xc_bf16.transpose(1, 0, 2)
    return arr


def kernel(x: np.ndarray, kernel: np.ndarray, _trace: bool = False):
    x = np.ascontiguousarray(x, dtype=np.float32)
    assert x.shape == (B, C, H, W), x.shape
    bands5, taps = _separable(kernel)
    bands_bf = bands5.astype(BF16)

    n_total = B * C
    n_per_core = n_total // N_CORES
    xb = x.reshape(n_total, H, W).astype(BF16)

    nc = _get_program(n_per_core, taps)
    in_maps = [
        {
            "x": _pack_input(xb[c * n_per_core : (c + 1) * n_per_core]),
            "bands": bands_bf,
        }
        for c in range(N_CORES)
    ]
    res = bass_utils.run_bass_kernel_spmd(
        nc, in_maps, core_ids=list(range(N_CORES)), trace=_trace
    )
    y = np.empty((n_total, H, W), dtype=np.float32)
    for c, r in enumerate(res.results):
        yc = np.asarray(r["y"]).reshape(H, n_per_core, W)
        y[c * n_per_core : (c + 1) * n_per_core] = yc.transpose(1, 0, 2).astype(
            np.float32
        )
    y = y.reshape(B, C, H, W)
    if _trace:
        return y, res
    return y


# revision 19
# speedup vs baseline: 3.2239x; 3.2239x over previous
"""Trainium2 Bass kernel for nn_Blur: depthwise 4x4 FIR blur (upfirdn2d pad=(2,1)).

Full inputs in, full output out. Internally shards the 4096 (b,c) images
across 8 NeuronCores (pure data parallel, no collectives).

v2 (bf16): tolerance is rel_err < 2e-2, so all device I/O is bf16 (host-side
RNE cast) — halves HBM traffic vs fp32 (memory-regime roofline win).  The
host also pre-arranges x into the exact SBUF layout the kernel wants
([h, img-major cols] with 2 zero gap columns per image), so every DMA is a
single large per-partition-contiguous transfer (~1.6 MB, 12.5 KB/partition
descriptors) instead of many 512 B-chunk strided ones.

Compute per core (512 images of [H=128, W=128]): the 4x4 depthwise conv
factors into 4 column-convolutions along H, each a banded matmul with the
contraction over the partition (H) axis, with the W-shift (j-2) realized as
a shifted moving-operand read of an accumulating matmul:
  psum[:, c] += W_j^T @ x[:, c + (j-2)]     W_j[hi, ho] = wf[hi-ho+2, j]
Images are packed at stride 130 (2 zero gap cols) so shifted reads pick up
zero padding at image edges.  Groups of 3 images share a PSUM bank; 4
groups (4 banks) form one chunk tile so PSUM->SBUF evacuation is one big
Vector/Scalar copy per chunk.
"""

import os
import sys
from contextlib import ExitStack

for _p in ("/opt/trn_rl_repo", "/root/.axon_site/_ro/trn_rl_repo"):
    if os.path.isdir(_p) and _p not in sys.path:
        sys.path.append(_p)

import ml_dtypes
import numpy as np

import concourse.bass as bass  # noqa: F401  (engine types referenced via nc)
import concourse.tile as tile
from concourse import bacc, bass_utils, mybir

BF16 = np.dtype(ml_dtypes.bfloat16)

B, C, H, W = 16, 256, 128, 128
N_CORES = 8
GROUP = 3          # images per PSUM bank / matmul group
STRIDE = 130       # 2-col gap + 128 data cols per image in the packed layout
PAD0 = 2           # upfirdn2d pad before (both spatial dims)
TILE = 24          # images per DMA tile (must be multiple of GROUP)
QG = 4             # PSUM banks (groups) per chunk tile
OFFLOAD_FRAC = 0.5  # fraction of full tiles W-conv'd on Vector/GpSimd (separable)

_PROGRAM_CACHE: dict[object, object] = {}


def _band_matrices(kern: np.ndarray) -> np.ndarray:
    """bands[j][hi, ho] = wf[hi-ho+2, j], wf = flip(kern). Shape [4,128,128]."""
    wf = np.flip(np.asarray(kern, dtype=np.float64), (0, 1))
    bands = np.zeros((4, H, H), dtype=np.float64)
    ho = np.arange(H)
    for j in range(4):
        for i in range(4):
            d = i - PAD0            # hi - ho
            hi = ho + d
            m = (hi >= 0) & (hi < H)
            bands[j][hi[m], ho[m]] = wf[i, j]
    return np.ascontiguousarray(bands.astype(np.float32))


def _tiles(n_images: int):
    """Split n_images into DMA tiles of at most TILE images.

    The first tiles ramp up small so the first matmul can start as soon as
    a small DMA lands; any ragged remainder goes LAST so the final
    output DMA (pure tail latency) is as small as possible.
    """
    ramp = [6, 6, 12]
    out = []
    i = 0
    for r in ramp:
        if n_images - i > r:
            out.append((i, r))
            i += r
    while i < n_images:
        n = min(TILE, n_images - i)
        out.append((i, n))
        i += n
    return out


def _groups(n_images: int):
    """Split a tile's images into matmul groups of at most GROUP, avoiding a
    trailing 1-image group (rebalance 3+1 -> 2+2)."""
    out = []
    i = 0
    while i < n_images:
        n = min(GROUP, n_images - i)
        out.append((i, n))
        i += n
    if len(out) >= 2 and out[-1][1] == 1:
        i0, n0 = out[-2]
        out[-2] = (i0, 2)
        out[-1] = (i0 + 2, 2)
    return out


def _offload_tiles(tiles, taps):
    """Pick which full tiles get the separable W-conv offload."""
    if taps is None or OFFLOAD_FRAC <= 0:
        return set()
    full = [i for i, (_, tn) in enumerate(tiles) if tn == TILE]
    if not full:
        return set()
    n_off = max(1, int(round(OFFLOAD_FRAC * len(full))))
    chosen = list(full[1::2][:n_off])
    for i in full[0::2]:
        if len(chosen) >= n_off:
            break
        chosen.append(i)
    return set(chosen)


def build_program(n_images: int, taps=None, xt_bufs: int = 6):
    """Build + compile the per-core Bass program for n_images [128,128] images.

    DRAM layout (host-prepared, bf16):
      x: [H, n_images*STRIDE + 2]  image k's column w at STRIDE*k + 2 + w,
         cols {STRIDE*k, STRIDE*k+1} and the trailing 2 are zeros.
      y: [H, n_images*W]           image k's column w at W*k + w.

    bands[0:4] are the fused HxW band matrices (4-pass path); bands[4] is
    the H-only band (offload path).  taps, when not None, are the 4 W-conv
    scalars (flipped W factor of the separable kernel) baked as immediates;
    tiles in _offload_tiles then run: PE H-conv (1 pass) -> PSUM -> SBUF
    evac (Scalar) -> 4-tap W-conv FMA chain (Vector/GpSimd alternating).
    """
    nc = bacc.Bacc("TRN2", target_bir_lowering=False, debug=False)
    f32 = mybir.dt.float32
    bf16 = mybir.dt.bfloat16

    x_d = nc.dram_tensor("x", [H, n_images * STRIDE + 2], bf16, kind="ExternalInput")
    b_d = nc.dram_tensor("bands", [5, H, H], bf16, kind="ExternalInput")
    y_d = nc.dram_tensor("y", [H, n_images * W], bf16, kind="ExternalOutput")

    tiles = _tiles(n_images)
    off_tiles = _offload_tiles(tiles, taps)

    with ExitStack() as ctx:
        tc = ctx.enter_context(tile.TileContext(nc))
        wpool = ctx.enter_context(tc.tile_pool(name="wpool", bufs=1))
        xpool = ctx.enter_context(tc.tile_pool(name="xpool", bufs=xt_bufs))
        opool = ctx.enter_context(tc.tile_pool(name="opool", bufs=4))
        tapool = ctx.enter_context(tc.tile_pool(name="tapool", bufs=2))
        wkpool = ctx.enter_context(tc.tile_pool(name="wkpool", bufs=6))
        ppool = ctx.enter_context(tc.tile_pool(name="ppool", bufs=2, space="PSUM"))

        wt = wpool.tile([H, 5 * H], bf16)
        nc.sync.dma_start(
            wt.rearrange("p (j b) -> p j b", b=H), b_d.rearrange("j a b -> a j b")
        )

        # Warm up the PE HAM clock gate with dummy matmuls on the weights
        # tile while the first input DMA is in flight: real matmuls then
        # start at 2.4 GHz instead of 1.2.
        warm = ppool.tile([H, 512 * QG], f32, tag="pt", name="pt")
        for _ in range(20):
            nc.tensor.matmul(
                warm[:, 0:256], wt[:, 0:H], wt[:, 0:256], start=True, stop=True
            )

        in_dma_engines = [nc.sync, nc.gpsimd]
        xts: dict[int, object] = {}

        def emit_in_dma(ti):
            i0, tn = tiles[ti]
            xt = xpool.tile([H, tn * STRIDE + 2], bf16, tag="xt", name="xt")
            in_dma_engines[ti % 2].dma_start(
                xt, x_d[:, i0 * STRIDE : i0 * STRIDE + tn * STRIDE + 2]
            )
            xts[ti] = xt

        for ti in range(min(4, len(tiles))):
            emit_in_dma(ti)

        def tA_view(tA, d, tn):
            """[p, tn, W] view of the gap-layout tile shifted by d columns."""
            span = tn * STRIDE
            if d <= 0:
                sl = tA[:, PAD0 + d : PAD0 + d + span]
                lo = 0
            else:
                sl = tA[:, PAD0 : PAD0 + span]
                lo = d
            return sl.rearrange("p (k c) -> p k c", c=STRIDE)[:, :, lo : lo + W]

        mult = mybir.AluOpType.mult
        add = mybir.AluOpType.add

        copy_idx = 0
        wconv_idx = 0
        for ti, (i0, tn) in enumerate(tiles):
            if ti + 4 < len(tiles):
                emit_in_dma(ti + 4)
            xt = xts.pop(ti)

            if ti in off_tiles:
                # --- separable offload path: PE does only the H-conv ---
                # W-conv runs on Vector as a mul/add chain: tensor_scalar is
                # 4x-mode (0.31 ns/elem, even strided-3D) and tensor_add 2x;
                # scalar_tensor_tensor never leaves 1x and GpSimd tensor ops
                # are 6-45us/op, so neither is used.
                wconv_idx += 1
                ot = opool.tile([H, tn * W], bf16, tag="ot", name="ot")
                span = tn * STRIDE + 2
                tA = tapool.tile([H, span], bf16, tag="ta", name="ta")
                c0 = 0
                while c0 < span:
                    cw = min(512 * QG, span - c0)
                    pt = ppool.tile([H, 512 * QG], f32, tag="pt", name="pt")
                    s = 0
                    while s < cw:
                        w_ = min(512, cw - s)
                        nc.tensor.matmul(
                            pt[:, s : s + w_],
                            wt[:, 4 * H : 5 * H],
                            xt[:, c0 + s : c0 + s + w_],
                            start=True,
                            stop=True,
                        )
                        s += w_
                    nc.scalar.copy(tA[:, c0 : c0 + cw], pt[:, 0:cw])
                    c0 += cw

                v = [tA_view(tA, d, tn) for d in (-2, -1, 0, 1)]
                e = nc.vector
                wk1 = wkpool.tile([H, tn * W], bf16, tag="wk", name="wk")
                wk2 = wkpool.tile([H, tn * W], bf16, tag="wk", name="wk")
                wk3 = wkpool.tile([H, tn * W], bf16, tag="wk", name="wk")
                w1v = wk1.rearrange("p (k c) -> p k c", c=W)
                w2v = wk2.rearrange("p (k c) -> p k c", c=W)
                w3v = wk3.rearrange("p (k c) -> p k c", c=W)
                otv = ot.rearrange("p (k c) -> p k c", c=W)
                if taps[0] == taps[3] and taps[1] == taps[2]:
                    e.tensor_add(w1v, v[0], v[3])
                    e.tensor_add(w2v, v[1], v[2])
                    e.tensor_scalar_mul(w3v, w1v, taps[0])
                    e.tensor_scalar_mul(w1v, w2v, taps[1])
                    e.tensor_add(otv, w3v, w1v)
                else:
                    e.tensor_scalar_mul(w1v, v[0], taps[0])
                    e.tensor_scalar_mul(w2v, v[1], taps[1])
                    e.tensor_add(w3v, w1v, w2v)
                    e.tensor_scalar_mul(w1v, v[2], taps[2])
                    e.tensor_add(w2v, w3v, w1v)
                    e.tensor_scalar_mul(w1v, v[3], taps[3])
                    e.tensor_add(otv, w2v, w1v)

                out_eng = (nc.sync, nc.scalar)[ti % 2]
                out_eng.dma_start(y_d[:, i0 * W : (i0 + tn) * W], ot)
                continue

            gs = _groups(tn)
            chunks = [gs[s : s + QG] for s in range(0, len(gs), QG)]
            ot = opool.tile([H, tn * W], bf16, tag="ot", name="ot")

            for chunk in chunks:
                nq = len(chunk)
                pt = ppool.tile([H, 512 * nq], f32, tag="pt", name="pt")
                # j-outer order amortizes the 4 stationary (band) loads over
                # the whole chunk; j=2 (d=0) first for the full-width
                # has_written-clearing write.
                for idx, j in enumerate((2, 0, 1, 3)):
                    d = j - PAD0
                    for q, (goff, n) in enumerate(chunk):
                        a = PAD0
                        b = STRIDE * n + PAD0 - (PAD0 if d > 0 else 0)
                        base = goff * STRIDE
                        nc.tensor.matmul(
                            pt[:, 512 * q + a : 512 * q + b],
                            wt[:, H * j : H * (j + 1)],
                            xt[:, base + a + d : base + b + d],
                            start=(idx == 0),
                            stop=(idx == 3),
                        )

                # PSUM -> SBUF evacuation (fp32 -> bf16).  One strided copy
                # per chunk when the chunk is uniform (all groups GROUP-sized);
                # per-group copies otherwise (ragged tail).  Scalar unless no
                # offloading is active (then alternate with Vector).
                uniform = all(n == GROUP for _, n in chunk)
                if off_tiles:
                    eng = (nc.scalar, nc.scalar)
                else:
                    eng = (nc.vector, nc.scalar)
                if uniform:
                    psrc = (
                        pt.rearrange("p (q c) -> p q c", c=512)[
                            :, :, : GROUP * STRIDE
                        ]
                        .rearrange("p q (k c) -> p q k c", c=STRIDE)[
                            :, :, :, PAD0 : PAD0 + W
                        ]
                    )
                    odst = ot[
                        :, chunk[0][0] * W : (chunk[-1][0] + GROUP) * W
                    ].rearrange("p (q k c) -> p q k c", q=nq, c=W)
                    e = eng[copy_idx % 2]
                    if e is nc.vector:
                        e.tensor_copy(odst, psrc)
                    else:
                        e.copy(odst, psrc)
                    copy_idx += 1
                else:
                    for q, (goff, n) in enumerate(chunk):
                        psrc = pt[:, 512 * q : 512 * q + STRIDE * n].rearrange(
                            "p (k c) -> p k c", c=STRIDE
                        )[:, :, PAD0 : PAD0 + W]
                        odst = ot[:, goff * W : (goff + n) * W].rearrange(
                            "p (k c) -> p k c", c=W
                        )
                        e = eng[copy_idx % 2]
                        if e is nc.vector:
                            e.tensor_copy(odst, psrc)
                        else:
                            e.copy(odst, psrc)
                        copy_idx += 1

            out_eng = (nc.sync, nc.scalar)[ti % 2]
            out_eng.dma_start(y_d[:, i0 * W : (i0 + tn) * W], ot)

    nc.compile()
    return nc


def _get_program(n_images: int, taps=None):
    key = (n_images, taps)
    if key not in _PROGRAM_CACHE:
        _PROGRAM_CACHE[key] = build_program(n_images, taps=taps)
    return _PROGRAM_CACHE[key]


def _separable(kern: np.ndarray):
    """Return (bands5_f32, taps) — taps None when kern is not rank-1."""
    K = np.asarray(kern, dtype=np.float64)
    bands5 = np.zeros((5, H, H), dtype=np.float32)
    bands5[0:4] = _band_matrices(kern)
    U, S, Vt = np.linalg.svd(K)
    if S[1] > 1e-6 * max(S[0], 1e-30):
        return bands5, None
    a = U[:, 0] * np.sqrt(S[0])
    b = Vt[0, :] * np.sqrt(S[0])
    af = a[::-1]  # flipped H factor
    bfl = b[::-1]  # flipped W factor -> the 4 free-dim taps
    ho = np.arange(H)
    Bh = np.zeros((H, H), dtype=np.float64)
    for i in range(4):
        hi = ho + (i - PAD0)
        m = (hi >= 0) & (hi < H)
        Bh[hi[m], ho[m]] = af[i]
    bands5[4] = Bh.astype(np.float32)
    taps = tuple(float(np.float32(v)) for v in bfl)
    return bands5, taps


def _pack_input(xc_bf16: np.ndarray) -> np.ndarray:
    """[n, H, W] bf16 -> [H, n*STRIDE + 2] bf16 gap layout."""
    n = xc_bf16.shape[0]
    arr = np.zeros((H, n * STRIDE + 2), dtype=BF16)
    v = np.lib.stride_tricks.as_strided(
        arr,
        shape=(H, n, STRIDE),
        strides=(arr.strides[0], STRIDE * arr.itemsize, arr.itemsize),
    )
    v[:, :, PAD0:] = xc_bf16.transpose(1, 0, 2)
    return arr


def kernel(x: np.ndarray, kernel: np.ndarray, _trace: bool = False):
    x = np.ascontiguousarray(x, dtype=np.float32)
    assert x.shape == (B, C, H, W), x.shape
    bands5, taps = _separable(kernel)
    bands_bf = bands5.astype(BF16)

    n_total = B * C
    n_per_core = n_total // N_CORES
    xb = x.reshape(n_total, H, W).astype(BF16)

    nc = _get_program(n_per_core, taps)
    in_maps = [
        {
            "x": _pack_input(xb[c * n_per_core : (c + 1) * n_per_core]),
            "bands": bands_bf,
        }
        for c in range(N_CORES)
    ]
    res = bass_utils.run_bass_kernel_spmd(
        nc, in_maps, core_ids=list(range(N_CORES)), trace=_trace
    )
    y = np.empty((n_total, H, W), dtype=np.float32)
    for c, r in enumerate(res.results):
        yc = np.asarray(r["y"]).reshape(H, n_per_core, W)
        y[c * n_per_core : (c + 1) * n_per_core] = yc.transpose(1, 0, 2).astype(
            np.float32
        )
    y = y.reshape(B, C, H, W)
    if _trace:
        return y, res
    return y
